# revision 7
# baseline (speedup 1.0000x reference)
# Multi-head attention (B=2, S=2048, D=1024, H=16) on 8 NeuronCores.
#
# Sharding: batch x head-group. Core c handles batch b = c//4 and heads
# 4*(c%4) .. 4*(c%4)+4 (4 heads = 256 head dims). Each core computes its
# heads' Q/K/V projections, causal attention, and a partial output
# projection through its slice of Wo; the host sums the 4 partials per
# batch (Wo row-parallel all-reduce done host-side).
#
# On-chip dataflow (per core):
#   qhT/khT = W @ x^T computed in [head_dim, seq] layout so the scores
#   matmul contracts head_dim on PE partitions; scores are produced
#   TRANSPOSED ([key, query]) so softmax normalization lands on the free
#   axis and P@V needs no transposes. The softmax denominator rides along
#   as a ones-column appended to V. exp runs on ScalarE with the padding
#   mask folded into the per-partition activation bias.
import sys
import numpy as np

for p in ("/opt/trn_rl_repo", "/opt/trn_rl_repo/concourse"):
    if p not in sys.path:
        sys.path.insert(0, p)

import ml_dtypes
import concourse.bass as bass
import concourse.mybir as mybir
import concourse.tile as tile
from concourse import bacc
from concourse import bass_utils

F32 = mybir.dt.float32
BF16 = mybir.dt.bfloat16

B, S, D, H = 2, 2048, 1024, 16
HD = D // H            # 64
N_CORES = 8
NH = 4                 # heads per core
DH = NH * HD           # 256 head dims per core
P = 128
QC = 512               # query chunk (free dim of score/PV matmuls)
NEG = -1.0e6


def build_nc(s=S, di=D, do=D, nh=NH, hd=HD):
    """Build the per-core Bass program (same program on all 8 cores)."""
    ni = di // P               # input-dim tiles
    nst = s // P               # seq tiles (also key tiles)
    nqc = s // QC              # query chunks
    tpq = QC // P              # key tiles per query chunk
    dh = nh * hd               # head dims on this core
    not_ = dh // P             # head-dim output tiles (head pairs)
    assert nh == 4 and hd == 64 and dh == 256 and not_ == 2

    nc = bacc.Bacc("TRN2", target_bir_lowering=False, debug=False)

    xq_d = nc.dram_tensor("xq_t", [di, s], BF16, kind="ExternalInput")
    xk_d = nc.dram_tensor("xk_t", [di, s], BF16, kind="ExternalInput")
    xv_d = nc.dram_tensor("xv_t", [di, s], BF16, kind="ExternalInput")
    wq_d = nc.dram_tensor("wq_t", [di, dh], BF16, kind="ExternalInput")
    wk_d = nc.dram_tensor("wk_t", [di, dh], BF16, kind="ExternalInput")
    wv_d = nc.dram_tensor("wv_t", [di, dh], BF16, kind="ExternalInput")
    wo_d = nc.dram_tensor("wo_t", [dh, do], BF16, kind="ExternalInput")
    pad_d = nc.dram_tensor("pad", [P, nst], F32, kind="ExternalInput")
    cau_d = nc.dram_tensor("causal", [P, P], F32, kind="ExternalInput")
    idn_d = nc.dram_tensor("ident", [P, P], BF16, kind="ExternalInput")
    out_d = nc.dram_tensor("out", [s, do], F32, kind="ExternalOutput")

    scale = 1.0 / float(np.sqrt(hd))

    from contextlib import ExitStack
    with tile.TileContext(nc) as tc, ExitStack() as ctx:
        wsb = ctx.enter_context(tc.tile_pool(name="wsb", bufs=1))
        xsb = ctx.enter_context(tc.tile_pool(name="xsb", bufs=3))
        hsb = ctx.enter_context(tc.tile_pool(name="hsb", bufs=1))
        ptp = ctx.enter_context(tc.tile_pool(name="ptp", bufs=6))
        avp = ctx.enter_context(tc.tile_pool(name="avp", bufs=9))
        stg = ctx.enter_context(tc.tile_pool(name="stg", bufs=2))
        osb = ctx.enter_context(tc.tile_pool(name="osb", bufs=3))
        ctxA = ctx.enter_context(ExitStack())
        proj_ps = ctxA.enter_context(tc.tile_pool(name="proj_ps", bufs=4, space="PSUM"))
        tr_ps = ctxA.enter_context(tc.tile_pool(name="tr_ps", bufs=2, space="PSUM"))

        # ---- constants / weights ----
        causal = wsb.tile([P, P], F32)
        nc.sync.dma_start(causal[:], cau_d[:])
        ident = wsb.tile([P, P], BF16)
        nc.sync.dma_start(ident[:], idn_d[:])
        pad = wsb.tile([P, nst], F32)
        nc.sync.dma_start(pad[:], pad_d[:])
        ones = wsb.tile([P, 64], F32)
        nc.vector.memset(ones[:], 1.0)

        wq = wsb.tile([P, ni, dh], BF16)
        wk = wsb.tile([P, ni, dh], BF16)
        wv = wsb.tile([P, ni, dh], BF16)
        nc.sync.dma_start(wq[:], wq_d.rearrange("(io p) o -> p io o", p=P))
        nc.sync.dma_start(wk[:], wk_d.rearrange("(io p) o -> p io o", p=P))
        nc.sync.dma_start(wv[:], wv_d.rearrange("(io p) o -> p io o", p=P))
        wo = wsb.tile([P, not_, do], BF16)
        nc.sync.dma_start(wo[:], wo_d.rearrange("(ct p) n -> p ct n", p=P))

        # ---- x loads (full tensors resident, bf16) ----
        xs = {}
        for name, dram in (("k", xk_d), ("q", xq_d), ("v", xv_d)):
            x = xsb.tile([P, ni, s], BF16, tag="x")
            xr = dram.rearrange("(io p) s -> p io s", p=P)
            for io in range(ni):
                nc.sync.dma_start(x[:, io, :], xr[:, io, :])
            xs[name] = x

        # ---- projections: {q,k,v}hT[o, s] = W[o,:] @ x^T ----
        qhT = hsb.tile([P, not_, s], BF16)
        khT = hsb.tile([P, not_, s], BF16)
        vhT = hsb.tile([P, not_, s], BF16)
        for dst, w, x in ((khT, wk, xs["k"]), (qhT, wq, xs["q"]), (vhT, wv, xs["v"])):
            for ot in range(not_):
                pss = [proj_ps.tile([P, QC], F32, tag="proj", name=f"proj{c}") for c in range(nqc)]
                for i in range(ni):
                    lhsT = w[:, i, ot * P:(ot + 1) * P]
                    for c in range(nqc):
                        nc.tensor.matmul(pss[c][:], lhsT, x[:, i, c * QC:(c + 1) * QC],
                                         start=(i == 0), stop=(i == ni - 1))
                for c in range(nqc):
                    nc.vector.tensor_copy(dst[:, ot, c * QC:(c + 1) * QC], pss[c][:])

        # ---- V transpose: vh[seq, head*65] with a ones column per head ----
        vh = hsb.tile([P, nst, nh * 65], BF16)
        ones_view = vh[:].rearrange("p t (h c) -> p t h c", c=65)[:, :, :, 64:65]
        nc.vector.memset(ones_view, 1.0)
        for ot in range(not_):
            for st in range(nst):
                tp = tr_ps.tile([P, P], BF16, tag="tr")
                nc.tensor.transpose(tp[:], vhT[:, ot, st * P:(st + 1) * P], ident[:])
                for j in range(2):
                    h = 2 * ot + j
                    nc.vector.tensor_copy(vh[:, st, h * 65:h * 65 + 64],
                                          tp[:, j * 64:(j + 1) * 64])

        ctxA.close()
        s_ps = ctx.enter_context(tc.tile_pool(name="s_ps", bufs=3, space="PSUM"))
        av_ps = ctx.enter_context(tc.tile_pool(name="av_ps", bufs=3, space="PSUM"))
        o_ps = ctx.enter_context(tc.tile_pool(name="o_ps", bufs=2, space="PSUM"))

        # ---- attention, by head pair ----
        aT = hsb.tile([P, not_, s], BF16)
        for hp in range(not_):
            av_raw = {}   # (h, c) -> [64, QC] f32 sbuf, unnormalized A^T chunk
            stage = {}    # h -> [P, QC] f32, sums at rows 32c
            for h in (2 * hp, 2 * hp + 1):
                stage[h] = stg.tile([P, 2, QC], F32, tag="stage", name=f"stage{h}")
                nc.vector.memset(stage[h][:], 1.0)
            for c in range(nqc):
                avs = {}
                for h in (2 * hp, 2 * hp + 1):
                    avs[h] = av_ps.tile([65, QC], F32, tag="av", name=f"av{h}")
                nt = min(nst, (c + 1) * tpq)
                for t in range(nt):
                    pts = {}
                    for h in (2 * hp, 2 * hp + 1):
                        hb = h % 2
                        rows = slice(hb * 64, (hb + 1) * 64)
                        sp = s_ps.tile([P, QC], F32, tag="s")
                        nc.tensor.matmul(sp[:], khT[rows, hp, t * P:(t + 1) * P],
                                         qhT[rows, hp, c * QC:(c + 1) * QC],
                                         start=True, stop=True)
                        j = t - c * tpq
                        start_col = max(0, j * P)
                        if j >= 0:  # diagonal tile: add causal mask on its triangle
                            nc.vector.tensor_tensor(
                                sp[:, j * P:(j + 1) * P], sp[:, j * P:(j + 1) * P],
                                causal[:], mybir.AluOpType.add)
                        pt = ptp.tile([P, QC], BF16, tag="pt")
                        nc.scalar.activation(pt[:, start_col:], sp[:, start_col:],
                                             mybir.ActivationFunctionType.Exp,
                                             bias=pad[:, t:t + 1], scale=scale)
                        pts[h] = (pt, start_col)
                    for h in (2 * hp, 2 * hp + 1):
                        pt, start_col = pts[h]
                        nc.tensor.matmul(avs[h][:, start_col:],
                                         vh[:, t, h * 65:(h + 1) * 65],
                                         pt[:, start_col:],
                                         start=(t == 0), stop=(t == nt - 1))
                for h in (2 * hp, 2 * hp + 1):
                    ar = avp.tile([64, QC], F32, tag="avraw")
                    nc.vector.tensor_copy(ar[:], avs[h][0:64, :])
                    av_raw[(h, c)] = ar
                    nc.scalar.copy(stage[h][64 * (c % 2):64 * (c % 2) + 1, c // 2, :],
                                   avs[h][64:65, :])
            for h in (2 * hp, 2 * hp + 1):
                rec = stg.tile([P, 2, QC], F32, tag="rec")
                nc.vector.reciprocal_approx_fast(rec[:], stage[h][:])
                hb = h % 2
                rows = slice(hb * 64, (hb + 1) * 64)
                for c in range(nqc):
                    rb = av_ps.tile([65, QC], F32, tag="av", name="rb")
                    r0 = 64 * (c % 2)
                    nc.tensor.matmul(rb[0:64, :], ones[r0:r0 + 1, 0:64],
                                     rec[r0:r0 + 1, c // 2, :], start=True, stop=True)
                    nc.vector.tensor_tensor(aT[rows, hp, c * QC:(c + 1) * QC],
                                            av_raw[(h, c)][:], rb[0:64, :],
                                            mybir.AluOpType.mult)

        # ---- output projection: out[s, do] = A^T.T @ WoT ----
        nno = do // 512
        for st in range(nst):
            ob = osb.tile([P, do], F32, tag="ob")
            pss = [o_ps.tile([P, 512], F32, tag="o", name=f"o{n}") for n in range(nno)]
            for ct in range(not_):
                lhsT = aT[:, ct, st * P:(st + 1) * P]
                for n in range(nno):
                    nc.tensor.matmul(pss[n][:], lhsT, wo[:, ct, n * 512:(n + 1) * 512],
                                     start=(ct == 0), stop=(ct == not_ - 1))
            for n in range(nno):
                nc.vector.tensor_copy(ob[:, n * 512:(n + 1) * 512], pss[n][:])
            nc.sync.dma_start(out_d[st * P:(st + 1) * P, :], ob[:])

    nc.compile()
    return nc


_NC_CACHE = {}


def _get_nc():
    key = (S, D)
    if key not in _NC_CACHE:
        _NC_CACHE[key] = build_nc()
    return _NC_CACHE[key]


def _bf(x):
    return np.ascontiguousarray(x).astype(ml_dtypes.bfloat16)


def kernel(q, k, v, attention_mask, Wq, Wk, Wv, Wo):
    q = np.asarray(q, np.float32)
    k = np.asarray(k, np.float32)
    v = np.asarray(v, np.float32)
    attention_mask = np.asarray(attention_mask)
    Wq, Wk, Wv, Wo = (np.asarray(w, np.float32) for w in (Wq, Wk, Wv, Wo))

    nst = S // P
    causal_np = np.where(np.arange(P)[None, :] >= np.arange(P)[:, None],
                         np.float32(0), np.float32(NEG)).astype(np.float32)
    ident_np = np.eye(P, dtype=np.float32).astype(ml_dtypes.bfloat16)

    xT = {}
    padb = {}
    for b in range(B):
        xT[b] = (_bf(q[b].T), _bf(k[b].T), _bf(v[b].T))
        padb[b] = np.ascontiguousarray(
            np.where(attention_mask[b] != 0, np.float32(0), np.float32(NEG))
            .astype(np.float32).reshape(nst, P).T)

    in_maps = []
    for c in range(N_CORES):
        b, hg = c // 4, c % 4
        rows = slice(hg * DH, (hg + 1) * DH)
        xq, xk, xv = xT[b]
        in_maps.append({
            "xq_t": xq, "xk_t": xk, "xv_t": xv,
            "wq_t": _bf(Wq[rows, :].T), "wk_t": _bf(Wk[rows, :].T),
            "wv_t": _bf(Wv[rows, :].T), "wo_t": _bf(Wo[:, rows].T),
            "pad": padb[b], "causal": causal_np, "ident": ident_np,
        })

    nc = _get_nc()
    res = bass_utils.run_bass_kernel_spmd(nc, in_maps, core_ids=list(range(N_CORES)))

    out = np.zeros((B, S, D), np.float32)
    for c in range(N_CORES):
        out[c // 4] += res.results[c]["out"]
    return out


# revision 9
# speedup vs baseline: 1.0317x; 1.0317x over previous
# Multi-head attention (B=2, S=2048, D=1024, H=16) on 8 NeuronCores.
#
# Sharding: batch x head-group. Core c handles batch b = c//4 and heads
# 4*(c%4) .. 4*(c%4)+4 (4 heads = 256 head dims). Each core computes its
# heads' Q/K/V projections, causal attention, and a partial output
# projection through its slice of Wo; the host sums the 4 partials per
# batch (Wo row-parallel all-reduce done host-side).
#
# On-chip dataflow (per core):
#   qhT/khT = W @ x^T computed in [head_dim, seq] layout so the scores
#   matmul contracts head_dim on PE partitions; scores are produced
#   TRANSPOSED ([key, query]) so softmax normalization lands on the free
#   axis and P@V needs no transposes. The softmax denominator rides along
#   as a ones-column appended to V. exp runs on ScalarE with the padding
#   mask folded into the per-partition activation bias. The two heads of
#   a pair live in the two PSUM banks of one score tile so exp/mask run
#   as single double-width instructions, and their K=64 matmuls land in
#   disjoint PE row groups (concurrent).
import sys
import numpy as np

for p in ("/opt/trn_rl_repo", "/opt/trn_rl_repo/concourse"):
    if p not in sys.path:
        sys.path.insert(0, p)

import ml_dtypes
import concourse.bass as bass
import concourse.mybir as mybir
import concourse.tile as tile
from concourse import bacc
from concourse import bass_utils

F32 = mybir.dt.float32
BF16 = mybir.dt.bfloat16

B, S, D, H = 2, 2048, 1024, 16
HD = D // H            # 64
N_CORES = 8
NH = 4                 # heads per core
DH = NH * HD           # 256 head dims per core
P = 128
QC = 512               # query chunk (free dim of score/PV matmuls)
NEG = -1.0e6


def build_nc(s=S, di=D, do=D, nh=NH, hd=HD):
    """Build the per-core Bass program (same program on all 8 cores)."""
    ni = di // P               # input-dim tiles
    nst = s // P               # seq tiles (also key tiles)
    nqc = s // QC              # query chunks
    tpq = QC // P              # key tiles per query chunk
    dh = nh * hd               # head dims on this core
    not_ = dh // P             # head-dim output tiles (head pairs)
    assert nh == 4 and hd == 64 and dh == 256 and not_ == 2

    nc = bacc.Bacc("TRN2", target_bir_lowering=False, debug=False)

    xq_d = nc.dram_tensor("xq_t", [di, s], BF16, kind="ExternalInput")
    xk_d = nc.dram_tensor("xk_t", [di, s], BF16, kind="ExternalInput")
    xv_d = nc.dram_tensor("xv_t", [di, s], BF16, kind="ExternalInput")
    wq_d = nc.dram_tensor("wq_t", [di, dh], BF16, kind="ExternalInput")
    wk_d = nc.dram_tensor("wk_t", [di, dh], BF16, kind="ExternalInput")
    wv_d = nc.dram_tensor("wv_t", [di, dh], BF16, kind="ExternalInput")
    wo_d = nc.dram_tensor("wo_t", [dh, do], BF16, kind="ExternalInput")
    pad_d = nc.dram_tensor("pad", [P, nst], F32, kind="ExternalInput")
    cau_d = nc.dram_tensor("causal", [P, P], F32, kind="ExternalInput")
    idn_d = nc.dram_tensor("ident", [P, P], BF16, kind="ExternalInput")
    out_d = nc.dram_tensor("out", [s, do], F32, kind="ExternalOutput")

    scale = 1.0 / float(np.sqrt(hd))

    from contextlib import ExitStack
    with tile.TileContext(nc) as tc, ExitStack() as ctx:
        wsb = ctx.enter_context(tc.tile_pool(name="wsb", bufs=1))
        xsb = ctx.enter_context(tc.tile_pool(name="xsb", bufs=3))
        hsb = ctx.enter_context(tc.tile_pool(name="hsb", bufs=1))
        ptp = ctx.enter_context(tc.tile_pool(name="ptp", bufs=4))
        avp = ctx.enter_context(tc.tile_pool(name="avp", bufs=9))
        stg = ctx.enter_context(tc.tile_pool(name="stg", bufs=2))
        osb = ctx.enter_context(tc.tile_pool(name="osb", bufs=3))
        ctxA = ctx.enter_context(ExitStack())
        proj_ps = ctxA.enter_context(tc.tile_pool(name="proj_ps", bufs=2, space="PSUM"))
        tr_ps = ctxA.enter_context(tc.tile_pool(name="tr_ps", bufs=2, space="PSUM"))

        # ---- constants / weights ----
        causal = wsb.tile([P, P], F32)
        nc.sync.dma_start(causal[:], cau_d[:])
        ident = wsb.tile([P, P], BF16)
        nc.sync.dma_start(ident[:], idn_d[:])
        pad = wsb.tile([P, nst], F32)
        nc.sync.dma_start(pad[:], pad_d[:])
        ones = wsb.tile([P, 64], F32)
        nc.vector.memset(ones[:], 1.0)

        wq = wsb.tile([P, ni, dh], BF16)
        wk = wsb.tile([P, ni, dh], BF16)
        wv = wsb.tile([P, ni, dh], BF16)
        nc.sync.dma_start(wq[:], wq_d.rearrange("(io p) o -> p io o", p=P))
        nc.sync.dma_start(wk[:], wk_d.rearrange("(io p) o -> p io o", p=P))
        nc.sync.dma_start(wv[:], wv_d.rearrange("(io p) o -> p io o", p=P))
        wo = wsb.tile([P, not_, do], BF16)
        nc.sync.dma_start(wo[:], wo_d.rearrange("(ct p) n -> p ct n", p=P))

        # ---- x loads (full tensors resident, bf16) ----
        xs = {}
        for name, dram in (("k", xk_d), ("q", xq_d), ("v", xv_d)):
            x = xsb.tile([P, ni, s], BF16, tag="x", name=f"x{name}")
            xr = dram.rearrange("(io p) s -> p io s", p=P)
            for io in range(ni):
                nc.sync.dma_start(x[:, io, :], xr[:, io, :])
            xs[name] = x

        # ---- projections: {q,k,v}hT[o, s] = W[o,:] @ x^T ----
        qhT = hsb.tile([P, not_, s], BF16)
        khT = hsb.tile([P, not_, s], BF16)
        vhT = hsb.tile([P, not_, s], BF16)
        ng = max(1, nqc // 2)  # psum groups of two query chunks
        for dst, w, x in ((khT, wk, xs["k"]), (qhT, wq, xs["q"]), (vhT, wv, xs["v"])):
            for ot in range(not_):
                pss = [proj_ps.tile([P, 2 * QC], F32, tag="proj", name=f"proj{g}")
                       for g in range(ng)]
                for i in range(ni):
                    lhsT = w[:, i, ot * P:(ot + 1) * P]
                    for c in range(nqc):
                        nc.tensor.matmul(
                            pss[c // 2][:, (c % 2) * QC:(c % 2 + 1) * QC], lhsT,
                            x[:, i, c * QC:(c + 1) * QC],
                            start=(i == 0), stop=(i == ni - 1))
                for g in range(ng):
                    nc.scalar.copy(dst[:, ot, g * 2 * QC:(g + 1) * 2 * QC], pss[g][:])

        # ---- V transpose: vh[seq, head*65] with a ones column per head ----
        vh = hsb.tile([P, nst, nh * 65], BF16)
        ones_view = vh[:].rearrange("p t (h c) -> p t h c", c=65)[:, :, :, 64:65]
        nc.vector.memset(ones_view, 1.0)
        for ot in range(not_):
            for st in range(nst):
                tp = tr_ps.tile([P, P], BF16, tag="tr")
                nc.tensor.transpose(tp[:], vhT[:, ot, st * P:(st + 1) * P], ident[:])
                dst3 = vh[:, st, 2 * ot * 65:(2 * ot + 2) * 65]
                dst3 = dst3.rearrange("p (h c) -> p h c", c=65)[:, :, 0:64]
                nc.vector.tensor_copy(dst3, tp[:].rearrange("p (h c) -> p h c", c=64))

        ctxA.close()
        s_ps = ctx.enter_context(tc.tile_pool(name="s_ps", bufs=2, space="PSUM"))
        av_ps = ctx.enter_context(tc.tile_pool(name="av_ps", bufs=2, space="PSUM"))
        o_ps = ctx.enter_context(tc.tile_pool(name="o_ps", bufs=2, space="PSUM"))

        # ---- attention, by head pair ----
        aT = hsb.tile([P, not_, s], BF16)
        for hp in range(not_):
            ha, hb_ = 2 * hp, 2 * hp + 1
            av_raw = {}   # (h, c) -> [64, QC] f32 sbuf, unnormalized A^T chunk
            stage = {}    # h -> [P, 2, QC] f32, sums at rows 64*(c%2), col c//2
            for h in (ha, hb_):
                stage[h] = stg.tile([P, 2, QC], F32, tag="stage", name=f"stage{h}")
                nc.vector.memset(stage[h][:], 1.0)
            for c in range(nqc):
                avs = {}
                for h in (ha, hb_):
                    avs[h] = av_ps.tile([65, QC], F32, tag="av", name=f"av{h}")
                nt = min(nst, (c + 1) * tpq)
                for t in range(nt):
                    sp = s_ps.tile([P, 2 * QC], F32, tag="s")
                    sp3 = sp[:].rearrange("p (h q) -> p h q", h=2)
                    for hh, h in enumerate((ha, hb_)):
                        rows = slice(hh * 64, (hh + 1) * 64)
                        nc.tensor.matmul(sp[:, hh * QC:(hh + 1) * QC],
                                         khT[rows, hp, t * P:(t + 1) * P],
                                         qhT[rows, hp, c * QC:(c + 1) * QC],
                                         start=True, stop=True)
                    j = t - c * tpq
                    start_col = max(0, j * P)
                    pt = ptp.tile([P, 2 * QC], BF16, tag="pt")
                    if j >= 0:  # diagonal tile: causal mask on both heads' triangles
                        nc.vector.tensor_tensor(
                            sp3[:, :, j * P:(j + 1) * P], sp3[:, :, j * P:(j + 1) * P],
                            causal[:, None, :].to_broadcast((P, 2, P)),
                            mybir.AluOpType.add)
                        pt3 = pt[:].rearrange("p (h q) -> p h q", h=2)
                        nc.scalar.activation(pt3[:, :, start_col:],
                                             sp3[:, :, start_col:],
                                             mybir.ActivationFunctionType.Exp,
                                             bias=pad[:, t:t + 1], scale=scale)
                    else:
                        nc.scalar.activation(pt[:], sp[:],
                                             mybir.ActivationFunctionType.Exp,
                                             bias=pad[:, t:t + 1], scale=scale)
                    for hh, h in enumerate((ha, hb_)):
                        nc.tensor.matmul(avs[h][:, start_col:],
                                         vh[:, t, h * 65:(h + 1) * 65],
                                         pt[:, hh * QC + start_col:(hh + 1) * QC],
                                         start=(t == 0), stop=(t == nt - 1))
                for h in (ha, hb_):
                    ar = avp.tile([64, QC], F32, tag="avraw")
                    nc.vector.tensor_copy(ar[:], avs[h][0:64, :])
                    av_raw[(h, c)] = ar
                    nc.vector.tensor_copy(
                        stage[h][64 * (c % 2):64 * (c % 2) + 1, c // 2, :],
                        avs[h][64:65, :])
            for h in (ha, hb_):
                rec = stg.tile([P, 2, QC], F32, tag="rec")
                nc.vector.reciprocal_approx_fast(rec[:], stage[h][:])
                hh = h % 2
                rows = slice(hh * 64, (hh + 1) * 64)
                for c in range(nqc):
                    rb = av_ps.tile([65, QC], F32, tag="av", name="rb")
                    r0 = 64 * (c % 2)
                    nc.tensor.matmul(rb[0:64, :], ones[r0:r0 + 1, 0:64],
                                     rec[r0:r0 + 1, c // 2, :], start=True, stop=True)
                    nc.vector.tensor_tensor(aT[rows, hp, c * QC:(c + 1) * QC],
                                            av_raw[(h, c)][:], rb[0:64, :],
                                            mybir.AluOpType.mult)

            # ---- this pair's partial output projection, accumulated into DRAM ----
            nno = do // 512
            for st in range(nst):
                ob = osb.tile([P, do], F32, tag="ob")
                lhsT = aT[:, hp, st * P:(st + 1) * P]
                for n in range(nno):
                    po = o_ps.tile([P, 512], F32, tag="o", name=f"o{n}")
                    nc.tensor.matmul(po[:], lhsT, wo[:, hp, n * 512:(n + 1) * 512],
                                     start=True, stop=True)
                    nc.scalar.copy(ob[:, n * 512:(n + 1) * 512], po[:])
                nc.gpsimd.dma_start(out_d[st * P:(st + 1) * P, :], ob[:],
                                    accum_op=mybir.AluOpType.add)

    nc.compile()
    return nc


_NC_CACHE = {}


def _get_nc():
    key = (S, D)
    if key not in _NC_CACHE:
        _NC_CACHE[key] = build_nc()
    return _NC_CACHE[key]


def _bf(x):
    return np.ascontiguousarray(x).astype(ml_dtypes.bfloat16)


def kernel(q, k, v, attention_mask, Wq, Wk, Wv, Wo):
    q = np.asarray(q, np.float32)
    k = np.asarray(k, np.float32)
    v = np.asarray(v, np.float32)
    attention_mask = np.asarray(attention_mask)
    Wq, Wk, Wv, Wo = (np.asarray(w, np.float32) for w in (Wq, Wk, Wv, Wo))

    nst = S // P
    causal_np = np.where(np.arange(P)[None, :] >= np.arange(P)[:, None],
                         np.float32(0), np.float32(NEG)).astype(np.float32)
    ident_np = np.eye(P, dtype=np.float32).astype(ml_dtypes.bfloat16)

    xT = {}
    padb = {}
    for b in range(B):
        xT[b] = (_bf(q[b].T), _bf(k[b].T), _bf(v[b].T))
        padb[b] = np.ascontiguousarray(
            np.where(attention_mask[b] != 0, np.float32(0), np.float32(NEG))
            .astype(np.float32).reshape(nst, P).T)

    in_maps = []
    for c in range(N_CORES):
        b, hg = c // 4, c % 4
        rows = slice(hg * DH, (hg + 1) * DH)
        xq, xk, xv = xT[b]
        in_maps.append({
            "xq_t": xq, "xk_t": xk, "xv_t": xv,
            "wq_t": _bf(Wq[rows, :].T), "wk_t": _bf(Wk[rows, :].T),
            "wv_t": _bf(Wv[rows, :].T), "wo_t": _bf(Wo[:, rows].T),
            "pad": padb[b], "causal": causal_np, "ident": ident_np,
        })

    nc = _get_nc()
    res = bass_utils.run_bass_kernel_spmd(nc, in_maps, core_ids=list(range(N_CORES)))

    out = np.zeros((B, S, D), np.float32)
    for c in range(N_CORES):
        out[c // 4] += res.results[c]["out"]
    return out
